# revision 28
# baseline (speedup 1.0000x reference)
"""Bass/Tile TRN2 kernel for LunarAttention (RoPE attention, B=2, S=2048, E=2048, H=16).

Sharding: 8 cores = 2 batches x 4 head-groups (4 heads / 512 dims each).
Per core (batch b, head-group g):
  Phase A: kT = rope(Wk_g hs^T + bk) in [d, s]; vS = hs Wv_g^T in [s, d]
           (v bias handled host-side: softmax weights sum to 1, so the bv
            term reduces to a constant bv_g @ Wo_g^T row added at gather).
  Phase B per 512-wide q chunk: q projection + rope, then per head
           scoresT[kv,q] -> exp (scalar engine) -> attn@v accumulation on PE.
           Softmax denominators via Pool-engine accumulation of exp tiles +
           partition_all_reduce (no PE ones-matmuls). Out-projection of the
           previous q chunk is interleaved into the attention loop.
dtypes: hs/W in bf16 (host-cast), attention math fp32/f32r, PSUM fp32.
"""

import math
import numpy as np

B, S, E, H, D = 2, 2048, 2048, 16, 128
G = 4            # head groups (cores per batch)
HPG = 4          # heads per group
F = HPG * D      # 512 dims per group
P = 128
EC = E // P      # 16 contraction chunks for projections
SB = 512         # phase-A s-block width
NSB = S // SB    # 4
QW = 512         # phase-B q chunk width
NQC = S // QW    # 4
KVC = S // P     # 16 kv chunks
NEC = 4          # out-proj e chunks of 512
SCALE = 1.0 / math.sqrt(D)

_NC_CACHE = {}


def _build_nc():
    import concourse.bass as bass
    import concourse.mybir as mybir
    import concourse.tile as tile
    from concourse import bacc
    from concourse import bass_isa
    from contextlib import ExitStack

    f32 = mybir.dt.float32
    f32r = mybir.dt.float32r
    bf16 = mybir.dt.bfloat16
    AF = mybir.ActivationFunctionType

    def r(ap):
        return ap.bitcast(f32r)

    nc = bacc.Bacc("TRN2", target_bir_lowering=False)

    hsT_d = nc.dram_tensor("hsT", [E, S], bf16, kind="ExternalInput")
    wqT_d = nc.dram_tensor("wqT", [E, F], bf16, kind="ExternalInput")
    wkT_d = nc.dram_tensor("wkT", [E, F], bf16, kind="ExternalInput")
    wvT_d = nc.dram_tensor("wvT", [E, F], bf16, kind="ExternalInput")
    woT_d = nc.dram_tensor("woT", [F, E], bf16, kind="ExternalInput")
    bq_d = nc.dram_tensor("bq2", [P, HPG], f32, kind="ExternalInput")
    bk_d = nc.dram_tensor("bk2", [P, HPG], f32, kind="ExternalInput")
    cos_d = nc.dram_tensor("cosT", [P, S], f32, kind="ExternalInput")
    sin_d = nc.dram_tensor("sinR", [P, S], f32, kind="ExternalInput")
    out_d = nc.dram_tensor("out", [S, E], f32, kind="ExternalOutput")

    hsT_r = hsT_d[:, :].rearrange("(c p) s -> p c s", p=P)      # [128, 16, 2048]
    wqT_r = wqT_d[:, :].rearrange("(c p) f -> p c f", p=P)
    wkT_r = wkT_d[:, :].rearrange("(c p) f -> p c f", p=P)
    wvT_r = wvT_d[:, :].rearrange("(c p) f -> p c f", p=P)
    woT_r = woT_d[:, :].rearrange("(c p) e -> p c e", p=P)      # [128, 4, 2048]

    with tile.TileContext(nc) as tc, ExitStack() as ctx:
        const = ctx.enter_context(tc.tile_pool(name="const", bufs=1))
        persist = ctx.enter_context(tc.tile_pool(name="persist", bufs=1))
        wpool = ctx.enter_context(tc.tile_pool(name="wpool", bufs=1))

        cos_sb = const.tile([P, S], f32, tag="cos")
        sin_sb = const.tile([P, S], f32, tag="sin")
        bq_sb = const.tile([P, HPG], f32, tag="bq")
        bk_sb = const.tile([P, HPG], f32, tag="bk")

        wq_sb = wpool.tile([P, EC, F], bf16, tag="wq")
        wk_sb = wpool.tile([P, EC, F], bf16, tag="wk")
        wv_sb = wpool.tile([P, EC, F], bf16, tag="wv")
        wo_sb = wpool.tile([P, HPG, E], bf16, tag="wo")

        # The first hs block and wk are the only DMAs the first K matmuls
        # wait on; all other loads are deferred to later s-blocks and issued
        # on the Activation DGE queue so DMA-engine arbitration cannot put
        # them ahead of the critical pair.
        def _load_trig_chunk(sb):
            # cos/sin consumers are per 512-wide block; loading per block
            # keeps the startup DMA window to ~1.5us of trig traffic.
            csl = slice(sb * SB, (sb + 1) * SB)
            nc.scalar.dma_start(out=cos_sb[:, csl], in_=cos_d[:, csl])
            nc.scalar.dma_start(out=sin_sb[:, csl], in_=sin_d[:, csl])

        def _deferred_consts_sb1():
            nc.scalar.dma_start(out=bq_sb[:, :], in_=bq_d[:, :])
            nc.scalar.dma_start(out=wq_sb[:, :, :], in_=wqT_r[:, :, :])
            nc.scalar.dma_start(out=wo_sb[:, :, :], in_=woT_r[:, :, :])

        kT = [persist.tile([P, S], f32r, tag=f"kT{h}", name=f"kT{h}")
              for h in range(HPG)]
        vS = [persist.tile([P, F], bf16, tag=f"vS{i}", name=f"vS{i}")
              for i in range(S // P)]

        def rope_halves(raw, dstT, dsl, psl, rp, sb, fc, nm):
            """dstT[:, dsl] = raw*cos[psl] + rotate_half(raw)*sin[psl], on DVE
            via cross-partition reads (no DMA swaps). sinTs rows 0:64 hold
            -sin, 64:128 hold +sin. DVE op cost scales with free size only,
            so the two half-partition muls cost the same as full ones."""
            w = dsl.stop - dsl.start
            ta = rp.tile([P, w], f32, tag=f"ta{nm}", name=f"ta_{nm}_{sb}_{fc}")
            tb = rp.tile([P, w], f32, tag=f"tb{nm}", name=f"tb_{nm}_{sb}_{fc}")
            nc.vector.tensor_mul(ta[:, :], raw[:, :], cos_sb[:, psl])
            # sin_sb holds sinTs rolled by 64 partitions so each mul's two
            # inputs share a base partition (HW verifier NCC_IBIR297); only
            # the output is partition-shifted.
            nc.vector.tensor_mul(tb[0:64, :], raw[64:128, :], sin_sb[64:128, psl])
            nc.vector.tensor_mul(tb[64:128, :], raw[0:64, :], sin_sb[0:64, psl])
            nc.vector.tensor_add(dstT[:, dsl], ta[:, :], tb[:, :])

        # Phase-B hs pool hoisted so chunk 0 can prefetch during phase A.
        hspB = ctx.enter_context(tc.tile_pool(name="hspB", bufs=1))

        def load_hsB(qc):
            qsl = slice(qc * QW, (qc + 1) * QW)
            hs_q = hspB.tile([P, EC, QW], bf16, tag="hsb", name=f"hsB_{qc}")
            nc.sync.dma_start(out=hs_q[:, :, :], in_=hsT_r[:, :, qsl])
            return hs_q

        # ---- Phase A: K (rope) + V projections ----
        with tc.tile_pool(name="hspA", bufs=2) as hspA, \
             tc.tile_pool(name="ppk", bufs=2, space="PSUM") as ppk, \
             tc.tile_pool(name="ppv", bufs=2, space="PSUM") as ppv, \
             tc.tile_pool(name="rpk", bufs=2) as rpk:
            for sb in range(NSB):
                ssl = slice(sb * SB, (sb + 1) * SB)
                hs_sb = hspA.tile([P, EC, SB], bf16, tag="hs", name=f"hsA_{sb}")
                if sb == 0:
                    # Quarter-granular interleaved loads: the first K matmul
                    # only waits for the first wk/hs chunk pair (~1.5us of
                    # DMA), not the full 4MB. Emission order must match
                    # producer->consumer order; the Act DGE queue keeps the
                    # const loads off the SP-queue semaphore gating the
                    # first K matmuls.
                    for cc in range(4):
                        ecs = slice(cc * 4, (cc + 1) * 4)
                        nc.sync.dma_start(out=wk_sb[:, ecs, :],
                                          in_=wkT_r[:, ecs, :])
                        nc.sync.dma_start(out=hs_sb[:, ecs, :],
                                          in_=hsT_r[:, ecs, ssl])
                        nc.scalar.dma_start(out=wv_sb[:, ecs, :],
                                            in_=wvT_r[:, ecs, :])
                    nc.scalar.dma_start(out=bk_sb[:, :], in_=bk_d[:, :])
                    _load_trig_chunk(0)
                else:
                    nc.sync.dma_start(out=hs_sb[:, :, :], in_=hsT_r[:, :, ssl])
                    _load_trig_chunk(sb)
                    if sb == 1:
                        _deferred_consts_sb1()
                for fc in range(HPG):
                    psK = ppk.tile([P, SB], f32, tag="psk", name=f"psk_{sb}_{fc}")
                    for ec in range(EC):
                        nc.tensor.matmul(
                            psK[:, :],
                            wk_sb[:, ec, fc * P:(fc + 1) * P],
                            hs_sb[:, ec, :],
                            start=(ec == 0),
                            stop=(ec == EC - 1),
                        )
                    raw = rpk.tile([P, SB], f32, tag="rawk", name=f"rawk_{sb}_{fc}")
                    nc.scalar.add(raw[:, :], psK[:, :], bk_sb[:, fc:fc + 1])
                    rope_halves(raw, kT[fc], ssl, ssl, rpk, sb, fc, "k")
                for m in range(SB // P):
                    psV = ppv.tile([P, F], f32, tag="psv", name=f"psv_{sb}_{m}")
                    for ec in range(EC):
                        nc.tensor.matmul(
                            psV[:, :],
                            hs_sb[:, ec, m * P:(m + 1) * P],
                            wv_sb[:, ec, :],
                            start=(ec == 0),
                            stop=(ec == EC - 1),
                        )
                    nc.scalar.copy(vS[sb * (SB // P) + m][:, :], psV[:, :])

        # ---- Phase B: Q proj + attention + interleaved out-projection ----
        # Schedule: per q chunk, the attention loop (64 kv iterations) has
        # two PE filler slots per iteration, fed by the previous chunk's
        # out-projection matmuls (64) and the next chunk's Q-projection
        # matmuls (64). This keeps PE fed past the Act-bound exp stream and
        # removes standalone projection phases where Act would idle.
        from concourse.bass_isa import ReduceOp
        hsb0 = load_hsB(0)
        with tc.tile_pool(name="qtp", bufs=2) as qtp, \
             tc.tile_pool(name="rpq", bufs=2) as rpq, \
             tc.tile_pool(name="psQ", bufs=2, space="PSUM") as psQ, \
             tc.tile_pool(name="psS", bufs=2, space="PSUM") as psS, \
             tc.tile_pool(name="psO", bufs=2, space="PSUM") as psO, \
             tc.tile_pool(name="psP", bufs=2, space="PSUM") as psP, \
             tc.tile_pool(name="exl", bufs=6) as exl, \
             tc.tile_pool(name="exs", bufs=2) as exs, \
             tc.tile_pool(name="rcp", bufs=2) as rcp, \
             tc.tile_pool(name="aop", bufs=2) as aop, \
             tc.tile_pool(name="otp", bufs=2) as otp:

            def qproj_gen(qc, hs_q, qT):
                """Yield once per Q-projection matmul for chunk qc; bias-add
                and rope are emitted inline as each head-part completes."""
                qsl = slice(qc * QW, (qc + 1) * QW)
                for fc in range(HPG):
                    ps_q = psQ.tile([P, QW], f32, tag="q", name=f"psq_{qc}_{fc}")
                    for ec in range(EC):
                        nc.tensor.matmul(
                            ps_q[:, :],
                            wq_sb[:, ec, fc * P:(fc + 1) * P],
                            hs_q[:, ec, :],
                            start=(ec == 0),
                            stop=(ec == EC - 1),
                        )
                        if ec < EC - 1:
                            yield
                    rawq = rpq.tile([P, QW], f32, tag="rawq", name=f"rawq_{qc}_{fc}")
                    nc.scalar.add(rawq[:, :], ps_q[:, :], bq_sb[:, fc:fc + 1])
                    rope_halves(rawq, qT[fc], slice(0, QW), qsl, rpq, qc, fc, "q")
                    yield

            def new_qT(qc):
                return [qtp.tile([P, QW], f32r, tag=f"qT{fc}", name=f"qT_{qc}_{fc}")
                        for fc in range(HPG)]

            def opj_gen(qc, ao):
                """Yield once per out-projection matmul for chunk qc
                (interleaved into the next chunk's attention loop)."""
                for ecn in range(NEC):
                    for m in range(QW // P):
                        pp = psP.tile([P, 512], f32, tag="pp",
                                      name=f"pp_{qc}_{ecn}_{m}")
                        for fc in range(HPG):
                            nc.tensor.matmul(
                                pp[:, :],
                                ao[fc][:, m * P:(m + 1) * P],
                                wo_sb[:, fc, ecn * 512:(ecn + 1) * 512],
                                start=(fc == 0), stop=(fc == HPG - 1),
                            )
                            if fc < HPG - 1:
                                yield
                        ot = otp.tile([P, 512], f32, tag="ot",
                                      name=f"ot_{qc}_{ecn}_{m}")
                        nc.vector.tensor_copy(ot[:, :], pp[:, :])
                        nc.sync.dma_start(
                            out=out_d[qc * QW + m * P: qc * QW + (m + 1) * P,
                                      ecn * 512:(ecn + 1) * 512],
                            in_=ot[:, :],
                        )
                        yield

            def opj_tail(qc, ao):
                """Un-interleaved tail out-projection: rotate pp tiles
                through all four (now idle) PSUM pools and defer each
                group's last matmul (which needs the final head's ao, whose
                normalization chain is still in flight) by 6 groups, so the
                PE never waits on it."""
                ppools = [(psP, "pp"), (psQ, "q"), (psO, "po"), (psS, "s")]
                opools = [(otp, "ot"), (rpq, "rawq"), (exs, "exsumA"),
                          (rcp, "sumb")]
                finish = []

                def close_group(item):
                    pp, ecn, m, use_act = item
                    nc.tensor.matmul(
                        pp[:, :],
                        ao[HPG - 1][:, m * P:(m + 1) * P],
                        wo_sb[:, HPG - 1, ecn * 512:(ecn + 1) * 512],
                        start=False, stop=True,
                    )
                    g = ecn * (QW // P) + m
                    opool, otag = opools[g % 4]
                    ot = opool.tile([P, 512], f32, tag=otag,
                                    name=f"otT_{qc}_{ecn}_{m}")
                    if use_act:
                        nc.scalar.copy(ot[:, :], pp[:, :])
                    else:
                        nc.vector.tensor_copy(ot[:, :], pp[:, :])
                    nc.sync.dma_start(
                        out=out_d[qc * QW + m * P: qc * QW + (m + 1) * P,
                                  ecn * 512:(ecn + 1) * 512],
                        in_=ot[:, :],
                    )

                for ecn in range(NEC):
                    for m in range(QW // P):
                        g = ecn * (QW // P) + m
                        pool, ptag = ppools[g % 4]
                        pp = pool.tile([P, 512], f32, tag=ptag,
                                       name=f"pp_{qc}_{ecn}_{m}")
                        for fc in range(HPG - 1):
                            nc.tensor.matmul(
                                pp[:, :],
                                ao[fc][:, m * P:(m + 1) * P],
                                wo_sb[:, fc, ecn * 512:(ecn + 1) * 512],
                                start=(fc == 0), stop=False,
                            )
                        finish.append((pp, ecn, m, g % 2 == 1))
                        if len(finish) > 6:
                            close_group(finish.pop(0))
                        if g >= 10 and finish:
                            close_group(finish.pop(0))
                for item in finish:
                    close_group(item)

            _DONE = object()

            def slot_list(opj, qpj):
                """128 lazy PE filler slots for one attention loop: the first
                16 slots draw from the out-projection stream only (lets the
                next hs block land before Q-projection starts), then the two
                streams alternate. Each fill() advances one generator by one
                matmul emission."""
                n_opj = 64 if opj is not None else 0
                n_qpj = 64 if qpj is not None else 0
                order = []
                o_left, q_left = n_opj, n_qpj
                for _ in range(16):
                    if o_left:
                        order.append(opj)
                        o_left -= 1
                while o_left or q_left:
                    if q_left:
                        order.append(qpj)
                        q_left -= 1
                    if o_left:
                        order.append(opj)
                        o_left -= 1
                return order

            def attention(qc, qT, slots):
                """Attention for chunk qc; two filler thunks per kv iter."""
                ao = [aop.tile([P, QW], mybir.dt.bfloat16, tag=f"ao{h}",
                               name=f"ao_{qc}_{h}") for h in range(HPG)]
                pending_norm = []
                si = iter(slots)

                def fill():
                    gen = next(si, None)
                    if gen is not None:
                        next(gen, None)

                def scores_mm(h, kv):
                    ps = psS.tile([P, QW], f32, tag="s", name=f"ps_{qc}_{h}_{kv}")
                    nc.tensor.matmul(
                        ps[:, :],
                        kT[h][:, kv * P:(kv + 1) * P],
                        qT[h][:, 0:QW],
                        start=True, stop=True,
                    )
                    ex = exl.tile([P, QW], bf16, tag="ex",
                                  name=f"ex_{qc}_{h}_{kv}")
                    nc.scalar.activation(ex[:, :], ps[:, :], AF.Exp, scale=SCALE)
                    return ex

                # One flat 64-iteration kv stream across all 4 heads with a
                # 2-deep exp pipeline crossing head boundaries, so PE never
                # waits on the first exps of a new head.
                po = exsumA = exsumB = None
                NIT = HPG * KVC
                exq = [scores_mm(0, 0), scores_mm(0, 1)]
                for hk in range(NIT):
                    h, kv = divmod(hk, KVC)
                    if kv == 0:
                        po = psO.tile([P, QW], f32, tag="po",
                                      name=f"po_{qc}_{h}")
                        # Two partial exp-sums: even kv chunks on DVE, odd
                        # on Pool, so neither engine gates the PE.
                        exsumA = exs.tile([P, QW], f32, tag="exsumA",
                                          name=f"exsumA_{qc}_{h}")
                        exsumB = exs.tile([P, QW], f32, tag="exsumB",
                                          name=f"exsumB_{qc}_{h}")
                    ex = exq.pop(0)
                    nc.tensor.matmul(
                        po[:, :],
                        vS[kv][:, h * D:(h + 1) * D],
                        ex[:, :],
                        start=(kv == 0), stop=(kv == KVC - 1),
                    )
                    if kv == 0:
                        nc.vector.tensor_copy(exsumA[:, :], ex[:, :])
                    elif kv == 1:
                        nc.gpsimd.tensor_copy(exsumB[:, :], ex[:, :])
                    elif kv % 2 == 0:
                        nc.vector.tensor_add(exsumA[:, :], exsumA[:, :],
                                             ex[:, :])
                    else:
                        nc.gpsimd.tensor_add(exsumB[:, :], exsumB[:, :],
                                             ex[:, :])
                    fill()
                    if hk + 2 < NIT:
                        h2, kv2 = divmod(hk + 2, KVC)
                        exq.append(scores_mm(h2, kv2))
                    fill()
                    if kv == 1 and pending_norm:
                        pending_norm.pop(0)()
                    if kv == KVC - 1:
                        def make_norm(h=h, po=po, exsumA=exsumA,
                                      exsumB=exsumB):
                            def _norm():
                                sumb = rcp.tile([P, QW], f32, tag="sumb",
                                                name=f"sumb_{qc}_{h}")
                                recb = rcp.tile([P, QW], f32, tag="recb",
                                                name=f"recb_{qc}_{h}")
                                nc.vector.tensor_add(
                                    exsumB[:, :], exsumB[:, :], exsumA[:, :])
                                nc.gpsimd.partition_all_reduce(
                                    sumb[:, :], exsumB[:, :], 128,
                                    ReduceOp.add)
                                nc.vector.reciprocal(recb[:, :], sumb[:, :])
                                nc.vector.tensor_mul(ao[h][:, :], po[:, :],
                                                     recb[:, :])
                            return _norm
                        pending_norm.append(make_norm())

                for fn in pending_norm:
                    fn()
                return ao

            # Prologue: Q projection for chunk 0 runs standalone.
            qT_cur = new_qT(0)
            for _ in qproj_gen(0, hsb0, qT_cur):
                pass
            hsb = load_hsB(1)

            prev_ao = None
            prev_qc = None
            for qc in range(NQC):
                opj = opj_gen(prev_qc, prev_ao) if prev_ao is not None else None
                if qc + 1 < NQC:
                    qT_next = new_qT(qc + 1)
                    qpj = qproj_gen(qc + 1, hsb, qT_next)
                else:
                    qT_next, qpj = None, None
                ao = attention(qc, qT_cur, slot_list(opj, qpj))
                prev_ao, prev_qc = ao, qc
                qT_cur = qT_next
                if qc + 2 < NQC:
                    hsb = load_hsB(qc + 2)
            # tail: out-projection of the last chunk
            opj_tail(prev_qc, prev_ao)

    nc.compile()
    return nc


def get_nc():
    if "nc" not in _NC_CACHE:
        _NC_CACHE["nc"] = _build_nc()
    return _NC_CACHE["nc"]


def _rope_tables():
    inv_freq = (1.0 / (10000.0 ** (np.arange(0, D, 2, dtype=np.float32) / np.float32(D)))).astype(np.float32)
    t = np.arange(S, dtype=np.float32)
    freqs = t[:, None] * inv_freq[None, :]               # [S, 64]
    emb = np.concatenate([freqs, freqs], axis=1)         # [S, 128]
    cosT = np.ascontiguousarray(np.cos(emb).T.astype(np.float32))      # [128, S]
    sin = np.sin(emb).astype(np.float32)                 # [S, 128]
    sinTs = np.ascontiguousarray(
        np.concatenate([-sin[:, :64], sin[:, 64:]], axis=1).T.astype(np.float32)
    )                                                    # [128, S]
    return cosT, sinTs


def _bf16(x):
    import ml_dtypes
    return np.ascontiguousarray(x).astype(ml_dtypes.bfloat16)


def make_in_maps(hidden_states, Wq, bq, Wk, bk, Wv, bv, Wo, bo):
    cosT, sinTs = _rope_tables()
    hsT = [_bf16(np.asarray(hidden_states[b], dtype=np.float32).T) for b in range(B)]
    in_maps = []
    for c in range(8):
        b, g = divmod(c, G)
        gs = slice(g * F, (g + 1) * F)
        in_maps.append({
            "hsT": hsT[b],
            "wqT": _bf16(np.asarray(Wq)[gs, :].T),
            "wkT": _bf16(np.asarray(Wk)[gs, :].T),
            "wvT": _bf16(np.asarray(Wv)[gs, :].T),
            "woT": _bf16(np.asarray(Wo)[:, gs].T),
            "bq2": np.ascontiguousarray(
                np.asarray(bq, dtype=np.float32)[gs].reshape(HPG, P).T),
            "bk2": np.ascontiguousarray(
                np.asarray(bk, dtype=np.float32)[gs].reshape(HPG, P).T),
            "cosT": cosT,
            "sinR": np.ascontiguousarray(np.roll(sinTs, 64, axis=0)),
        })
    return in_maps


def assemble_output(results, bv, Wo, bo):
    out = np.zeros((B, S, E), dtype=np.float32)
    for c in range(8):
        b = c // G
        out[b] += results[c]["out"]
    # v-bias folded out on device: softmax rows sum to 1, so the bv term is
    # the constant row bv @ Wo^T; add it with bo here.
    const_row = (np.asarray(bv, dtype=np.float32)
                 @ np.asarray(Wo, dtype=np.float32).T
                 + np.asarray(bo, dtype=np.float32))
    out += const_row[None, None, :]
    return out


def run_with_results(inputs, trace=False, **trace_kwargs):
    from concourse.bass_utils import run_bass_kernel_spmd
    nc = get_nc()
    in_maps = make_in_maps(**inputs)
    res = run_bass_kernel_spmd(nc, in_maps, list(range(8)), trace=trace, **trace_kwargs)
    out = assemble_output(res.results, inputs["bv"], inputs["Wo"], inputs["bo"])
    return out, res


def kernel(**inputs):
    out, _ = run_with_results(inputs)
    return out


# revision 29
# speedup vs baseline: 1.0338x; 1.0338x over previous
"""Bass/Tile TRN2 kernel for LunarAttention (RoPE attention, B=2, S=2048, E=2048, H=16).

Sharding: 8 cores = 2 batches x 4 head-groups (4 heads / 512 dims each).
Per core (batch b, head-group g):
  Phase A: kT = rope(Wk_g hs^T + bk) in [d, s]; vS = hs Wv_g^T in [s, d]
           (v bias handled host-side: softmax weights sum to 1, so the bv
            term reduces to a constant bv_g @ Wo_g^T row added at gather).
  Phase B per 512-wide q chunk: q projection + rope, then per head
           scoresT[kv,q] -> exp (scalar engine) -> attn@v accumulation on PE.
           Softmax denominators via Pool-engine accumulation of exp tiles +
           partition_all_reduce (no PE ones-matmuls). Out-projection of the
           previous q chunk is interleaved into the attention loop.
dtypes: hs/W in bf16 (host-cast), attention math fp32/f32r, PSUM fp32.
"""

import math
import numpy as np

B, S, E, H, D = 2, 2048, 2048, 16, 128
G = 4            # head groups (cores per batch)
HPG = 4          # heads per group
F = HPG * D      # 512 dims per group
P = 128
EC = E // P      # 16 contraction chunks for projections
SB = 512         # phase-A s-block width
NSB = S // SB    # 4
QW = 512         # phase-B q chunk width
NQC = S // QW    # 4
KVC = S // P     # 16 kv chunks
NEC = 4          # out-proj e chunks of 512
SCALE = 1.0 / math.sqrt(D)

_NC_CACHE = {}


def _build_nc():
    import concourse.bass as bass
    import concourse.mybir as mybir
    import concourse.tile as tile
    from concourse import bacc
    from concourse import bass_isa
    from contextlib import ExitStack

    f32 = mybir.dt.float32
    f32r = mybir.dt.float32r
    bf16 = mybir.dt.bfloat16
    AF = mybir.ActivationFunctionType

    def r(ap):
        return ap.bitcast(f32r)

    nc = bacc.Bacc("TRN2", target_bir_lowering=False)

    hsT_d = nc.dram_tensor("hsT", [E, S], bf16, kind="ExternalInput")
    wqT_d = nc.dram_tensor("wqT", [E, F], bf16, kind="ExternalInput")
    wkT_d = nc.dram_tensor("wkT", [E, F], bf16, kind="ExternalInput")
    wvT_d = nc.dram_tensor("wvT", [E, F], bf16, kind="ExternalInput")
    woT_d = nc.dram_tensor("woT", [F, E], bf16, kind="ExternalInput")
    bq_d = nc.dram_tensor("bq2", [P, HPG], f32, kind="ExternalInput")
    bk_d = nc.dram_tensor("bk2", [P, HPG], f32, kind="ExternalInput")
    cos_d = nc.dram_tensor("cosT", [P, S], f32, kind="ExternalInput")
    sin_d = nc.dram_tensor("sinR", [P, S], f32, kind="ExternalInput")
    out_d = nc.dram_tensor("out", [S, E], f32, kind="ExternalOutput")

    hsT_r = hsT_d[:, :].rearrange("(c p) s -> p c s", p=P)      # [128, 16, 2048]
    wqT_r = wqT_d[:, :].rearrange("(c p) f -> p c f", p=P)
    wkT_r = wkT_d[:, :].rearrange("(c p) f -> p c f", p=P)
    wvT_r = wvT_d[:, :].rearrange("(c p) f -> p c f", p=P)
    woT_r = woT_d[:, :].rearrange("(c p) e -> p c e", p=P)      # [128, 4, 2048]

    with tile.TileContext(nc) as tc, ExitStack() as ctx:
        const = ctx.enter_context(tc.tile_pool(name="const", bufs=1))
        persist = ctx.enter_context(tc.tile_pool(name="persist", bufs=1))
        wpool = ctx.enter_context(tc.tile_pool(name="wpool", bufs=1))

        cos_sb = const.tile([P, S], f32, tag="cos")
        sin_sb = const.tile([P, S], f32, tag="sin")
        bq_sb = const.tile([P, HPG], f32, tag="bq")
        bk_sb = const.tile([P, HPG], f32, tag="bk")

        wq_sb = wpool.tile([P, EC, F], bf16, tag="wq")
        wk_sb = wpool.tile([P, EC, F], bf16, tag="wk")
        wv_sb = wpool.tile([P, EC, F], bf16, tag="wv")
        wo_sb = wpool.tile([P, HPG, E], bf16, tag="wo")

        # The first hs block and wk are the only DMAs the first K matmuls
        # wait on; all other loads are deferred to later s-blocks and issued
        # on the Activation DGE queue so DMA-engine arbitration cannot put
        # them ahead of the critical pair.
        def _load_trig_chunk(sb):
            # cos/sin consumers are per 512-wide block; loading per block
            # keeps the startup DMA window to ~1.5us of trig traffic.
            csl = slice(sb * SB, (sb + 1) * SB)
            nc.scalar.dma_start(out=cos_sb[:, csl], in_=cos_d[:, csl])
            nc.scalar.dma_start(out=sin_sb[:, csl], in_=sin_d[:, csl])

        def _deferred_consts_sb1():
            nc.scalar.dma_start(out=bq_sb[:, :], in_=bq_d[:, :])
            nc.scalar.dma_start(out=wq_sb[:, :, :], in_=wqT_r[:, :, :])
            nc.scalar.dma_start(out=wo_sb[:, :, :], in_=woT_r[:, :, :])

        kT = [persist.tile([P, S], f32r, tag=f"kT{h}", name=f"kT{h}")
              for h in range(HPG)]
        vS = [persist.tile([P, F], bf16, tag=f"vS{i}", name=f"vS{i}")
              for i in range(S // P)]

        def rope_halves(raw, dstT, dsl, psl, rp, sb, fc, nm):
            """dstT[:, dsl] = raw*cos[psl] + rotate_half(raw)*sin[psl], on DVE
            via cross-partition reads (no DMA swaps). sinTs rows 0:64 hold
            -sin, 64:128 hold +sin. DVE op cost scales with free size only,
            so the two half-partition muls cost the same as full ones."""
            w = dsl.stop - dsl.start
            ta = rp.tile([P, w], f32, tag=f"ta{nm}", name=f"ta_{nm}_{sb}_{fc}")
            tb = rp.tile([P, w], f32, tag=f"tb{nm}", name=f"tb_{nm}_{sb}_{fc}")
            nc.vector.tensor_mul(ta[:, :], raw[:, :], cos_sb[:, psl])
            # sin_sb holds sinTs rolled by 64 partitions so each mul's two
            # inputs share a base partition (HW verifier NCC_IBIR297); only
            # the output is partition-shifted.
            nc.vector.tensor_mul(tb[0:64, :], raw[64:128, :], sin_sb[64:128, psl])
            nc.vector.tensor_mul(tb[64:128, :], raw[0:64, :], sin_sb[0:64, psl])
            nc.vector.tensor_add(dstT[:, dsl], ta[:, :], tb[:, :])

        # Phase-B hs pool hoisted so chunk 0 can prefetch during phase A.
        hspB = ctx.enter_context(tc.tile_pool(name="hspB", bufs=1))

        def load_hsB(qc):
            qsl = slice(qc * QW, (qc + 1) * QW)
            hs_q = hspB.tile([P, EC, QW], bf16, tag="hsb", name=f"hsB_{qc}")
            nc.sync.dma_start(out=hs_q[:, :, :], in_=hsT_r[:, :, qsl])
            return hs_q

        # ---- Phase A: K (rope) + V projections ----
        with tc.tile_pool(name="hspA", bufs=2) as hspA, \
             tc.tile_pool(name="ppk", bufs=2, space="PSUM") as ppk, \
             tc.tile_pool(name="ppv", bufs=2, space="PSUM") as ppv, \
             tc.tile_pool(name="rpk", bufs=2) as rpk:
            for sb in range(NSB):
                ssl = slice(sb * SB, (sb + 1) * SB)
                hs_sb = hspA.tile([P, EC, SB], bf16, tag="hs", name=f"hsA_{sb}")
                if sb == 0:
                    # Quarter-granular interleaved loads: the first K matmul
                    # only waits for the first wk/hs chunk pair (~1.5us of
                    # DMA), not the full 4MB. Emission order must match
                    # producer->consumer order; the Act DGE queue keeps the
                    # const loads off the SP-queue semaphore gating the
                    # first K matmuls.
                    for cc in range(4):
                        ecs = slice(cc * 4, (cc + 1) * 4)
                        nc.sync.dma_start(out=wk_sb[:, ecs, :],
                                          in_=wkT_r[:, ecs, :])
                        nc.sync.dma_start(out=hs_sb[:, ecs, :],
                                          in_=hsT_r[:, ecs, ssl])
                    nc.scalar.dma_start(out=bk_sb[:, :], in_=bk_d[:, :])
                    _load_trig_chunk(0)
                else:
                    nc.sync.dma_start(out=hs_sb[:, :, :], in_=hsT_r[:, :, ssl])
                    _load_trig_chunk(sb)
                    if sb == 1:
                        _deferred_consts_sb1()
                for fc in range(HPG):
                    psK = ppk.tile([P, SB], f32, tag="psk", name=f"psk_{sb}_{fc}")
                    for ec in range(EC):
                        nc.tensor.matmul(
                            psK[:, :],
                            wk_sb[:, ec, fc * P:(fc + 1) * P],
                            hs_sb[:, ec, :],
                            start=(ec == 0),
                            stop=(ec == EC - 1),
                        )
                    raw = rpk.tile([P, SB], f32, tag="rawk", name=f"rawk_{sb}_{fc}")
                    nc.scalar.add(raw[:, :], psK[:, :], bk_sb[:, fc:fc + 1])
                    rope_halves(raw, kT[fc], ssl, ssl, rpk, sb, fc, "k")
                if sb == 0:
                    nc.scalar.dma_start(out=wv_sb[:, :, :], in_=wvT_r[:, :, :])
                for m in range(SB // P):
                    psV = ppv.tile([P, F], f32, tag="psv", name=f"psv_{sb}_{m}")
                    for ec in range(EC):
                        nc.tensor.matmul(
                            psV[:, :],
                            hs_sb[:, ec, m * P:(m + 1) * P],
                            wv_sb[:, ec, :],
                            start=(ec == 0),
                            stop=(ec == EC - 1),
                        )
                    nc.scalar.copy(vS[sb * (SB // P) + m][:, :], psV[:, :])

        # ---- Phase B: Q proj + attention + interleaved out-projection ----
        # Schedule: per q chunk, the attention loop (64 kv iterations) has
        # two PE filler slots per iteration, fed by the previous chunk's
        # out-projection matmuls (64) and the next chunk's Q-projection
        # matmuls (64). This keeps PE fed past the Act-bound exp stream and
        # removes standalone projection phases where Act would idle.
        from concourse.bass_isa import ReduceOp
        hsb0 = load_hsB(0)
        with tc.tile_pool(name="qtp", bufs=2) as qtp, \
             tc.tile_pool(name="rpq", bufs=2) as rpq, \
             tc.tile_pool(name="psQ", bufs=2, space="PSUM") as psQ, \
             tc.tile_pool(name="psS", bufs=2, space="PSUM") as psS, \
             tc.tile_pool(name="psO", bufs=2, space="PSUM") as psO, \
             tc.tile_pool(name="psP", bufs=2, space="PSUM") as psP, \
             tc.tile_pool(name="exl", bufs=6) as exl, \
             tc.tile_pool(name="exs", bufs=2) as exs, \
             tc.tile_pool(name="rcp", bufs=2) as rcp, \
             tc.tile_pool(name="aop", bufs=2) as aop, \
             tc.tile_pool(name="otp", bufs=2) as otp:

            def qproj_gen(qc, hs_q, qT):
                """Yield once per Q-projection matmul for chunk qc; bias-add
                and rope are emitted inline as each head-part completes."""
                qsl = slice(qc * QW, (qc + 1) * QW)
                for fc in range(HPG):
                    ps_q = psQ.tile([P, QW], f32, tag="q", name=f"psq_{qc}_{fc}")
                    for ec in range(EC):
                        nc.tensor.matmul(
                            ps_q[:, :],
                            wq_sb[:, ec, fc * P:(fc + 1) * P],
                            hs_q[:, ec, :],
                            start=(ec == 0),
                            stop=(ec == EC - 1),
                        )
                        if ec < EC - 1:
                            yield
                    rawq = rpq.tile([P, QW], f32, tag="rawq", name=f"rawq_{qc}_{fc}")
                    nc.scalar.add(rawq[:, :], ps_q[:, :], bq_sb[:, fc:fc + 1])
                    rope_halves(rawq, qT[fc], slice(0, QW), qsl, rpq, qc, fc, "q")
                    yield

            def new_qT(qc):
                return [qtp.tile([P, QW], f32r, tag=f"qT{fc}", name=f"qT_{qc}_{fc}")
                        for fc in range(HPG)]

            def opj_gen(qc, ao):
                """Yield once per out-projection matmul for chunk qc
                (interleaved into the next chunk's attention loop)."""
                for ecn in range(NEC):
                    for m in range(QW // P):
                        pp = psP.tile([P, 512], f32, tag="pp",
                                      name=f"pp_{qc}_{ecn}_{m}")
                        for fc in range(HPG):
                            nc.tensor.matmul(
                                pp[:, :],
                                ao[fc][:, m * P:(m + 1) * P],
                                wo_sb[:, fc, ecn * 512:(ecn + 1) * 512],
                                start=(fc == 0), stop=(fc == HPG - 1),
                            )
                            if fc < HPG - 1:
                                yield
                        ot = otp.tile([P, 512], f32, tag="ot",
                                      name=f"ot_{qc}_{ecn}_{m}")
                        nc.vector.tensor_copy(ot[:, :], pp[:, :])
                        nc.sync.dma_start(
                            out=out_d[qc * QW + m * P: qc * QW + (m + 1) * P,
                                      ecn * 512:(ecn + 1) * 512],
                            in_=ot[:, :],
                        )
                        yield

            def opj_tail(qc, ao):
                """Un-interleaved tail out-projection: rotate pp tiles
                through all four (now idle) PSUM pools and defer each
                group's last matmul (which needs the final head's ao, whose
                normalization chain is still in flight) by 6 groups, so the
                PE never waits on it."""
                ppools = [(psP, "pp"), (psQ, "q"), (psO, "po"), (psS, "s")]
                opools = [(otp, "ot"), (rpq, "rawq"), (exs, "exsumA"),
                          (rcp, "sumb")]
                finish = []

                def close_group(item):
                    pp, ecn, m, use_act = item
                    nc.tensor.matmul(
                        pp[:, :],
                        ao[HPG - 1][:, m * P:(m + 1) * P],
                        wo_sb[:, HPG - 1, ecn * 512:(ecn + 1) * 512],
                        start=False, stop=True,
                    )
                    g = ecn * (QW // P) + m
                    opool, otag = opools[g % 4]
                    ot = opool.tile([P, 512], f32, tag=otag,
                                    name=f"otT_{qc}_{ecn}_{m}")
                    if use_act:
                        nc.scalar.copy(ot[:, :], pp[:, :])
                    else:
                        nc.vector.tensor_copy(ot[:, :], pp[:, :])
                    nc.sync.dma_start(
                        out=out_d[qc * QW + m * P: qc * QW + (m + 1) * P,
                                  ecn * 512:(ecn + 1) * 512],
                        in_=ot[:, :],
                    )

                for ecn in range(NEC):
                    for m in range(QW // P):
                        g = ecn * (QW // P) + m
                        pool, ptag = ppools[g % 4]
                        pp = pool.tile([P, 512], f32, tag=ptag,
                                       name=f"pp_{qc}_{ecn}_{m}")
                        for fc in range(HPG - 1):
                            nc.tensor.matmul(
                                pp[:, :],
                                ao[fc][:, m * P:(m + 1) * P],
                                wo_sb[:, fc, ecn * 512:(ecn + 1) * 512],
                                start=(fc == 0), stop=False,
                            )
                        finish.append((pp, ecn, m, g % 2 == 1))
                        if len(finish) > 6:
                            close_group(finish.pop(0))
                        if g >= 10 and finish:
                            close_group(finish.pop(0))
                for item in finish:
                    close_group(item)

            _DONE = object()

            def slot_list(opj, qpj):
                """128 lazy PE filler slots for one attention loop: the first
                16 slots draw from the out-projection stream only (lets the
                next hs block land before Q-projection starts), then the two
                streams alternate. Each fill() advances one generator by one
                matmul emission."""
                n_opj = 64 if opj is not None else 0
                n_qpj = 64 if qpj is not None else 0
                order = []
                o_left, q_left = n_opj, n_qpj
                for _ in range(16):
                    if o_left:
                        order.append(opj)
                        o_left -= 1
                while o_left or q_left:
                    if q_left:
                        order.append(qpj)
                        q_left -= 1
                    if o_left:
                        order.append(opj)
                        o_left -= 1
                return order

            def attention(qc, qT, slots):
                """Attention for chunk qc; two filler thunks per kv iter."""
                ao = [aop.tile([P, QW], mybir.dt.bfloat16, tag=f"ao{h}",
                               name=f"ao_{qc}_{h}") for h in range(HPG)]
                pending_norm = []
                si = iter(slots)

                def fill():
                    gen = next(si, None)
                    if gen is not None:
                        next(gen, None)

                def scores_mm(h, kv):
                    ps = psS.tile([P, QW], f32, tag="s", name=f"ps_{qc}_{h}_{kv}")
                    nc.tensor.matmul(
                        ps[:, :],
                        kT[h][:, kv * P:(kv + 1) * P],
                        qT[h][:, 0:QW],
                        start=True, stop=True,
                    )
                    ex = exl.tile([P, QW], bf16, tag="ex",
                                  name=f"ex_{qc}_{h}_{kv}")
                    nc.scalar.activation(ex[:, :], ps[:, :], AF.Exp, scale=SCALE)
                    return ex

                # One flat 64-iteration kv stream across all 4 heads with a
                # 2-deep exp pipeline crossing head boundaries, so PE never
                # waits on the first exps of a new head.
                po = exsumA = exsumB = None
                NIT = HPG * KVC
                exq = [scores_mm(0, 0), scores_mm(0, 1)]
                for hk in range(NIT):
                    h, kv = divmod(hk, KVC)
                    if kv == 0:
                        po = psO.tile([P, QW], f32, tag="po",
                                      name=f"po_{qc}_{h}")
                        # Two partial exp-sums: even kv chunks on DVE, odd
                        # on Pool, so neither engine gates the PE.
                        exsumA = exs.tile([P, QW], f32, tag="exsumA",
                                          name=f"exsumA_{qc}_{h}")
                        exsumB = exs.tile([P, QW], f32, tag="exsumB",
                                          name=f"exsumB_{qc}_{h}")
                    ex = exq.pop(0)
                    nc.tensor.matmul(
                        po[:, :],
                        vS[kv][:, h * D:(h + 1) * D],
                        ex[:, :],
                        start=(kv == 0), stop=(kv == KVC - 1),
                    )
                    if kv == 0:
                        nc.vector.tensor_copy(exsumA[:, :], ex[:, :])
                    elif kv == 1:
                        nc.gpsimd.tensor_copy(exsumB[:, :], ex[:, :])
                    elif kv % 2 == 0:
                        nc.vector.tensor_add(exsumA[:, :], exsumA[:, :],
                                             ex[:, :])
                    else:
                        nc.gpsimd.tensor_add(exsumB[:, :], exsumB[:, :],
                                             ex[:, :])
                    fill()
                    if hk + 2 < NIT:
                        h2, kv2 = divmod(hk + 2, KVC)
                        exq.append(scores_mm(h2, kv2))
                    fill()
                    if kv == 1 and pending_norm:
                        pending_norm.pop(0)()
                    if kv == KVC - 1:
                        def make_norm(h=h, po=po, exsumA=exsumA,
                                      exsumB=exsumB):
                            def _norm():
                                sumb = rcp.tile([P, QW], f32, tag="sumb",
                                                name=f"sumb_{qc}_{h}")
                                recb = rcp.tile([P, QW], f32, tag="recb",
                                                name=f"recb_{qc}_{h}")
                                nc.vector.tensor_add(
                                    exsumB[:, :], exsumB[:, :], exsumA[:, :])
                                nc.gpsimd.partition_all_reduce(
                                    sumb[:, :], exsumB[:, :], 128,
                                    ReduceOp.add)
                                nc.vector.reciprocal(recb[:, :], sumb[:, :])
                                nc.vector.tensor_mul(ao[h][:, :], po[:, :],
                                                     recb[:, :])
                            return _norm
                        pending_norm.append(make_norm())

                for fn in pending_norm:
                    fn()
                return ao

            # Prologue: Q projection for chunk 0 runs standalone.
            qT_cur = new_qT(0)
            for _ in qproj_gen(0, hsb0, qT_cur):
                pass
            hsb = load_hsB(1)

            prev_ao = None
            prev_qc = None
            for qc in range(NQC):
                opj = opj_gen(prev_qc, prev_ao) if prev_ao is not None else None
                if qc + 1 < NQC:
                    qT_next = new_qT(qc + 1)
                    qpj = qproj_gen(qc + 1, hsb, qT_next)
                else:
                    qT_next, qpj = None, None
                ao = attention(qc, qT_cur, slot_list(opj, qpj))
                prev_ao, prev_qc = ao, qc
                qT_cur = qT_next
                if qc + 2 < NQC:
                    hsb = load_hsB(qc + 2)
            # tail: out-projection of the last chunk
            opj_tail(prev_qc, prev_ao)

    nc.compile()
    return nc


def get_nc():
    if "nc" not in _NC_CACHE:
        _NC_CACHE["nc"] = _build_nc()
    return _NC_CACHE["nc"]


def _rope_tables():
    inv_freq = (1.0 / (10000.0 ** (np.arange(0, D, 2, dtype=np.float32) / np.float32(D)))).astype(np.float32)
    t = np.arange(S, dtype=np.float32)
    freqs = t[:, None] * inv_freq[None, :]               # [S, 64]
    emb = np.concatenate([freqs, freqs], axis=1)         # [S, 128]
    cosT = np.ascontiguousarray(np.cos(emb).T.astype(np.float32))      # [128, S]
    sin = np.sin(emb).astype(np.float32)                 # [S, 128]
    sinTs = np.ascontiguousarray(
        np.concatenate([-sin[:, :64], sin[:, 64:]], axis=1).T.astype(np.float32)
    )                                                    # [128, S]
    return cosT, sinTs


def _bf16(x):
    import ml_dtypes
    return np.ascontiguousarray(x).astype(ml_dtypes.bfloat16)


def make_in_maps(hidden_states, Wq, bq, Wk, bk, Wv, bv, Wo, bo):
    cosT, sinTs = _rope_tables()
    hsT = [_bf16(np.asarray(hidden_states[b], dtype=np.float32).T) for b in range(B)]
    in_maps = []
    for c in range(8):
        b, g = divmod(c, G)
        gs = slice(g * F, (g + 1) * F)
        in_maps.append({
            "hsT": hsT[b],
            "wqT": _bf16(np.asarray(Wq)[gs, :].T),
            "wkT": _bf16(np.asarray(Wk)[gs, :].T),
            "wvT": _bf16(np.asarray(Wv)[gs, :].T),
            "woT": _bf16(np.asarray(Wo)[:, gs].T),
            "bq2": np.ascontiguousarray(
                np.asarray(bq, dtype=np.float32)[gs].reshape(HPG, P).T),
            "bk2": np.ascontiguousarray(
                np.asarray(bk, dtype=np.float32)[gs].reshape(HPG, P).T),
            "cosT": cosT,
            "sinR": np.ascontiguousarray(np.roll(sinTs, 64, axis=0)),
        })
    return in_maps


def assemble_output(results, bv, Wo, bo):
    out = np.zeros((B, S, E), dtype=np.float32)
    for c in range(8):
        b = c // G
        out[b] += results[c]["out"]
    # v-bias folded out on device: softmax rows sum to 1, so the bv term is
    # the constant row bv @ Wo^T; add it with bo here.
    const_row = (np.asarray(bv, dtype=np.float32)
                 @ np.asarray(Wo, dtype=np.float32).T
                 + np.asarray(bo, dtype=np.float32))
    out += const_row[None, None, :]
    return out


def run_with_results(inputs, trace=False, **trace_kwargs):
    from concourse.bass_utils import run_bass_kernel_spmd
    nc = get_nc()
    in_maps = make_in_maps(**inputs)
    res = run_bass_kernel_spmd(nc, in_maps, list(range(8)), trace=trace, **trace_kwargs)
    out = assemble_output(res.results, inputs["bv"], inputs["Wo"], inputs["bo"])
    return out, res


def kernel(**inputs):
    out, _ = run_with_results(inputs)
    return out


# revision 32
# speedup vs baseline: 1.0588x; 1.0241x over previous
"""Bass/Tile TRN2 kernel for LunarAttention (RoPE attention, B=2, S=2048, E=2048, H=16).

Sharding: 8 cores = 2 batches x 4 head-groups (4 heads / 512 dims each).
Per core (batch b, head-group g):
  Phase A: kT = rope(Wk_g hs^T + bk) in [d, s]; vS = hs Wv_g^T in [s, d]
           (v bias handled host-side: softmax weights sum to 1, so the bv
            term reduces to a constant bv_g @ Wo_g^T row added at gather).
  Phase B per 512-wide q chunk: q projection + rope, then per head
           scoresT[kv,q] -> exp (scalar engine) -> attn@v accumulation on PE.
           Softmax denominators via Pool-engine accumulation of exp tiles +
           partition_all_reduce (no PE ones-matmuls). Out-projection of the
           previous q chunk is interleaved into the attention loop.
dtypes: hs/W in bf16 (host-cast), attention math fp32/f32r, PSUM fp32.
"""

import math
import numpy as np

B, S, E, H, D = 2, 2048, 2048, 16, 128
G = 4            # head groups (cores per batch)
HPG = 4          # heads per group
F = HPG * D      # 512 dims per group
P = 128
EC = E // P      # 16 contraction chunks for projections
SB = 512         # phase-A s-block width
NSB = S // SB    # 4
QW = 512         # phase-B q chunk width
NQC = S // QW    # 4
KVC = S // P     # 16 kv chunks
NEC = 4          # out-proj e chunks of 512
SCALE = 1.0 / math.sqrt(D)

_NC_CACHE = {}


def _build_nc():
    import concourse.bass as bass
    import concourse.mybir as mybir
    import concourse.tile as tile
    from concourse import bacc
    from concourse import bass_isa
    from contextlib import ExitStack

    f32 = mybir.dt.float32
    f32r = mybir.dt.float32r
    bf16 = mybir.dt.bfloat16
    AF = mybir.ActivationFunctionType

    def r(ap):
        return ap.bitcast(f32r)

    nc = bacc.Bacc("TRN2", target_bir_lowering=False)

    hsT_d = nc.dram_tensor("hsT", [E, S], bf16, kind="ExternalInput")
    wqT_d = nc.dram_tensor("wqT", [E, F], bf16, kind="ExternalInput")
    wkT_d = nc.dram_tensor("wkT", [E, F], bf16, kind="ExternalInput")
    wvT_d = nc.dram_tensor("wvT", [E, F], bf16, kind="ExternalInput")
    woT_d = nc.dram_tensor("woT", [F, E], bf16, kind="ExternalInput")
    bq_d = nc.dram_tensor("bq2", [P, HPG], f32, kind="ExternalInput")
    bk_d = nc.dram_tensor("bk2", [P, HPG], f32, kind="ExternalInput")
    cos_d = nc.dram_tensor("cosT", [P, S], f32, kind="ExternalInput")
    sin_d = nc.dram_tensor("sinR", [P, S], f32, kind="ExternalInput")
    out_d = nc.dram_tensor("out", [S, E], f32, kind="ExternalOutput")

    hsT_r = hsT_d[:, :].rearrange("(c p) s -> p c s", p=P)      # [128, 16, 2048]
    wqT_r = wqT_d[:, :].rearrange("(c p) f -> p c f", p=P)
    wkT_r = wkT_d[:, :].rearrange("(c p) f -> p c f", p=P)
    wvT_r = wvT_d[:, :].rearrange("(c p) f -> p c f", p=P)
    woT_r = woT_d[:, :].rearrange("(c p) e -> p c e", p=P)      # [128, 4, 2048]

    with tile.TileContext(nc) as tc, ExitStack() as ctx:
        const = ctx.enter_context(tc.tile_pool(name="const", bufs=1))
        persist = ctx.enter_context(tc.tile_pool(name="persist", bufs=1))
        wpool = ctx.enter_context(tc.tile_pool(name="wpool", bufs=1))

        cos_sb = const.tile([P, S], f32, tag="cos")
        sin_sb = const.tile([P, S], f32, tag="sin")
        bq_sb = const.tile([P, HPG], f32, tag="bq")
        bk_sb = const.tile([P, HPG], f32, tag="bk")

        wq_sb = wpool.tile([P, EC, F], bf16, tag="wq")
        wk_sb = wpool.tile([P, EC, F], bf16, tag="wk")
        wv_sb = wpool.tile([P, EC, F], bf16, tag="wv")
        wo_sb = wpool.tile([P, HPG, E], bf16, tag="wo")

        # The first hs block and wk are the only DMAs the first K matmuls
        # wait on; all other loads are deferred to later s-blocks and issued
        # on the Activation DGE queue so DMA-engine arbitration cannot put
        # them ahead of the critical pair.
        def _load_trig_chunk(sb):
            # cos/sin consumers are per 512-wide block; loading per block
            # keeps the startup DMA window to ~1.5us of trig traffic.
            csl = slice(sb * SB, (sb + 1) * SB)
            nc.scalar.dma_start(out=cos_sb[:, csl], in_=cos_d[:, csl])
            nc.scalar.dma_start(out=sin_sb[:, csl], in_=sin_d[:, csl])

        def _deferred_consts_sb1():
            nc.scalar.dma_start(out=bq_sb[:, :], in_=bq_d[:, :])
            nc.scalar.dma_start(out=wq_sb[:, :, :], in_=wqT_r[:, :, :])
            nc.scalar.dma_start(out=wo_sb[:, :, :], in_=woT_r[:, :, :])

        kT = [persist.tile([P, S], f32r, tag=f"kT{h}", name=f"kT{h}")
              for h in range(HPG)]
        vS = [persist.tile([P, F], bf16, tag=f"vS{i}", name=f"vS{i}")
              for i in range(S // P)]

        def rope_halves(raw, dstT, dsl, psl, rp, sb, fc, nm):
            """dstT[:, dsl] = raw*cos[psl] + rotate_half(raw)*sin[psl], on DVE
            via cross-partition reads (no DMA swaps). sinTs rows 0:64 hold
            -sin, 64:128 hold +sin. DVE op cost scales with free size only,
            so the two half-partition muls cost the same as full ones."""
            w = dsl.stop - dsl.start
            ta = rp.tile([P, w], f32, tag=f"ta{nm}", name=f"ta_{nm}_{sb}_{fc}")
            tb = rp.tile([P, w], f32, tag=f"tb{nm}", name=f"tb_{nm}_{sb}_{fc}")
            nc.vector.tensor_mul(ta[:, :], raw[:, :], cos_sb[:, psl])
            # sin_sb holds sinTs rolled by 64 partitions so each mul's two
            # inputs share a base partition (HW verifier NCC_IBIR297); only
            # the output is partition-shifted.
            nc.vector.tensor_mul(tb[0:64, :], raw[64:128, :], sin_sb[64:128, psl])
            nc.vector.tensor_mul(tb[64:128, :], raw[0:64, :], sin_sb[0:64, psl])
            nc.vector.tensor_add(dstT[:, dsl], ta[:, :], tb[:, :])

        # Phase-B hs pool hoisted so chunk 0 can prefetch during phase A.
        hspB = ctx.enter_context(tc.tile_pool(name="hspB", bufs=1))

        def load_hsB(qc):
            qsl = slice(qc * QW, (qc + 1) * QW)
            hs_q = hspB.tile([P, EC, QW], bf16, tag="hsb", name=f"hsB_{qc}")
            nc.sync.dma_start(out=hs_q[:, :, :], in_=hsT_r[:, :, qsl])
            return hs_q

        # ---- Phase A: K (rope) + V projections ----
        with tc.tile_pool(name="hspA", bufs=2) as hspA, \
             tc.tile_pool(name="ppk", bufs=2, space="PSUM") as ppk, \
             tc.tile_pool(name="ppv", bufs=2, space="PSUM") as ppv, \
             tc.tile_pool(name="rpk", bufs=2) as rpk:
            for sb in range(NSB):
                ssl = slice(sb * SB, (sb + 1) * SB)
                hs_sb = hspA.tile([P, EC, SB], bf16, tag="hs", name=f"hsA_{sb}")
                if sb == 0:
                    # Quarter-granular interleaved loads: the first K matmul
                    # only waits for the first wk/hs chunk pair (~1.5us of
                    # DMA), not the full 4MB. Emission order must match
                    # producer->consumer order; the Act DGE queue keeps the
                    # const loads off the SP-queue semaphore gating the
                    # first K matmuls.
                    for cc in range(4):
                        ecs = slice(cc * 4, (cc + 1) * 4)
                        nc.sync.dma_start(out=wk_sb[:, ecs, :],
                                          in_=wkT_r[:, ecs, :])
                        nc.sync.dma_start(out=hs_sb[:, ecs, :],
                                          in_=hsT_r[:, ecs, ssl])
                    nc.scalar.dma_start(out=bk_sb[:, :], in_=bk_d[:, :])
                    _load_trig_chunk(0)
                else:
                    nc.sync.dma_start(out=hs_sb[:, :, :], in_=hsT_r[:, :, ssl])
                    _load_trig_chunk(sb)
                    if sb == 1:
                        _deferred_consts_sb1()
                for fc in range(HPG):
                    psK = ppk.tile([P, SB], f32, tag="psk", name=f"psk_{sb}_{fc}")
                    for ec in range(EC):
                        nc.tensor.matmul(
                            psK[:, :],
                            wk_sb[:, ec, fc * P:(fc + 1) * P],
                            hs_sb[:, ec, :],
                            start=(ec == 0),
                            stop=(ec == EC - 1),
                        )
                    raw = rpk.tile([P, SB], f32, tag="rawk", name=f"rawk_{sb}_{fc}")
                    nc.scalar.add(raw[:, :], psK[:, :], bk_sb[:, fc:fc + 1])
                    rope_halves(raw, kT[fc], ssl, ssl, rpk, sb, fc, "k")
                if sb == 0:
                    nc.scalar.dma_start(out=wv_sb[:, :, :], in_=wvT_r[:, :, :])
                for m in range(SB // P):
                    psV = ppv.tile([P, F], f32, tag="psv", name=f"psv_{sb}_{m}")
                    for ec in range(EC):
                        nc.tensor.matmul(
                            psV[:, :],
                            hs_sb[:, ec, m * P:(m + 1) * P],
                            wv_sb[:, ec, :],
                            start=(ec == 0),
                            stop=(ec == EC - 1),
                        )
                    nc.scalar.copy(vS[sb * (SB // P) + m][:, :], psV[:, :])

        # ---- Phase B: Q proj + attention + interleaved out-projection ----
        # Schedule: per q chunk, the attention loop (64 kv iterations) has
        # two PE filler slots per iteration, fed by the previous chunk's
        # out-projection matmuls (64) and the next chunk's Q-projection
        # matmuls (64). This keeps PE fed past the Act-bound exp stream and
        # removes standalone projection phases where Act would idle.
        from concourse.bass_isa import ReduceOp
        hsb0 = load_hsB(0)
        with tc.tile_pool(name="qtp", bufs=2) as qtp, \
             tc.tile_pool(name="rpq", bufs=2) as rpq, \
             tc.tile_pool(name="psQ", bufs=2, space="PSUM") as psQ, \
             tc.tile_pool(name="psS", bufs=2, space="PSUM") as psS, \
             tc.tile_pool(name="psO", bufs=2, space="PSUM") as psO, \
             tc.tile_pool(name="psP", bufs=2, space="PSUM") as psP, \
             tc.tile_pool(name="exl", bufs=6) as exl, \
             tc.tile_pool(name="exs", bufs=2) as exs, \
             tc.tile_pool(name="rcp", bufs=2) as rcp, \
             tc.tile_pool(name="aop", bufs=2) as aop, \
             tc.tile_pool(name="otp", bufs=2) as otp:

            def qproj_gen(qc, hs_q, qT):
                """Yield once per Q-projection matmul for chunk qc; bias-add
                and rope are emitted inline as each head-part completes."""
                qsl = slice(qc * QW, (qc + 1) * QW)
                for fc in range(HPG):
                    ps_q = psQ.tile([P, QW], f32, tag="q", name=f"psq_{qc}_{fc}")
                    for ec in range(EC):
                        nc.tensor.matmul(
                            ps_q[:, :],
                            wq_sb[:, ec, fc * P:(fc + 1) * P],
                            hs_q[:, ec, :],
                            start=(ec == 0),
                            stop=(ec == EC - 1),
                        )
                        if ec < EC - 1:
                            yield
                    rawq = rpq.tile([P, QW], f32, tag="rawq", name=f"rawq_{qc}_{fc}")
                    nc.scalar.add(rawq[:, :], ps_q[:, :], bq_sb[:, fc:fc + 1])
                    rope_halves(rawq, qT[fc], slice(0, QW), qsl, rpq, qc, fc, "q")
                    yield

            def new_qT(qc):
                return [qtp.tile([P, QW], f32r, tag=f"qT{fc}", name=f"qT_{qc}_{fc}")
                        for fc in range(HPG)]

            def opj_gen(qc, ao):
                """Yield once per out-projection matmul for chunk qc
                (interleaved into the next chunk's attention loop)."""
                for ecn in range(NEC):
                    for m in range(QW // P):
                        pp = psP.tile([P, 512], f32, tag="pp",
                                      name=f"pp_{qc}_{ecn}_{m}")
                        for fc in range(HPG):
                            nc.tensor.matmul(
                                pp[:, :],
                                ao[fc][:, m * P:(m + 1) * P],
                                wo_sb[:, fc, ecn * 512:(ecn + 1) * 512],
                                start=(fc == 0), stop=(fc == HPG - 1),
                            )
                            if fc < HPG - 1:
                                yield
                        ot = otp.tile([P, 512], f32, tag="ot",
                                      name=f"ot_{qc}_{ecn}_{m}")
                        nc.vector.tensor_copy(ot[:, :], pp[:, :])
                        nc.sync.dma_start(
                            out=out_d[qc * QW + m * P: qc * QW + (m + 1) * P,
                                      ecn * 512:(ecn + 1) * 512],
                            in_=ot[:, :],
                        )
                        yield

            def opj_tail(qc, ao):
                """Un-interleaved tail out-projection: rotate pp tiles
                through all four (now idle) PSUM pools and defer each
                group's last matmul (which needs the final head's ao, whose
                normalization chain is still in flight) by 6 groups, so the
                PE never waits on it."""
                ppools = [(psP, "pp"), (psQ, "q"), (psO, "po"), (psS, "s")]
                opools = [(otp, "ot"), (rpq, "rawq"), (exs, "exsumA"),
                          (rcp, "sumb")]
                finish = []

                def close_group(item):
                    pp, ecn, m, use_act = item
                    nc.tensor.matmul(
                        pp[:, :],
                        ao[HPG - 1][:, m * P:(m + 1) * P],
                        wo_sb[:, HPG - 1, ecn * 512:(ecn + 1) * 512],
                        start=False, stop=True,
                    )
                    g = ecn * (QW // P) + m
                    opool, otag = opools[g % 4]
                    ot = opool.tile([P, 512], f32, tag=otag,
                                    name=f"otT_{qc}_{ecn}_{m}")
                    if use_act:
                        nc.scalar.copy(ot[:, :], pp[:, :])
                    else:
                        nc.vector.tensor_copy(ot[:, :], pp[:, :])
                    nc.sync.dma_start(
                        out=out_d[qc * QW + m * P: qc * QW + (m + 1) * P,
                                  ecn * 512:(ecn + 1) * 512],
                        in_=ot[:, :],
                    )

                for ecn in range(NEC):
                    for m in range(QW // P):
                        g = ecn * (QW // P) + m
                        pool, ptag = ppools[g % 4]
                        pp = pool.tile([P, 512], f32, tag=ptag,
                                       name=f"pp_{qc}_{ecn}_{m}")
                        for fc in range(HPG - 1):
                            nc.tensor.matmul(
                                pp[:, :],
                                ao[fc][:, m * P:(m + 1) * P],
                                wo_sb[:, fc, ecn * 512:(ecn + 1) * 512],
                                start=(fc == 0), stop=False,
                            )
                        finish.append((pp, ecn, m, g % 2 == 1))
                        if len(finish) > 6:
                            close_group(finish.pop(0))
                        if g >= 10 and finish:
                            close_group(finish.pop(0))
                for item in finish:
                    close_group(item)

            _DONE = object()

            def slot_list(opj, qpj):
                """128 lazy PE filler slots for one attention loop: the first
                16 slots draw from the out-projection stream only (lets the
                next hs block land before Q-projection starts), then the two
                streams alternate. Each fill() advances one generator by one
                matmul emission."""
                if opj is not None and qpj is not None:
                    order = []
                    o_left, q_left = 64, 64
                    for _ in range(16):
                        order.append(opj)
                        o_left -= 1
                    while o_left or q_left:
                        if q_left:
                            order.append(qpj)
                            q_left -= 1
                        if o_left:
                            order.append(opj)
                            o_left -= 1
                    return order
                # Single-stream chunks (first: no out-projection; last: no
                # next Q-projection): spread the 64 fills one per iteration,
                # otherwise the last 32 iterations run bare and the PE
                # starves behind the Act exp stream.
                gen = opj if opj is not None else qpj
                if gen is None:
                    return []
                return [gen, None] * 64

            def attention(qc, qT, slots):
                """Attention for chunk qc; two filler thunks per kv iter."""
                ao = [aop.tile([P, QW], mybir.dt.bfloat16, tag=f"ao{h}",
                               name=f"ao_{qc}_{h}") for h in range(HPG)]
                pending_norm = []
                si = iter(slots)

                def fill():
                    gen = next(si, None)
                    if gen is not None:
                        next(gen, None)

                def scores_mm(h, kv):
                    ps = psS.tile([P, QW], f32, tag="s", name=f"ps_{qc}_{h}_{kv}")
                    nc.tensor.matmul(
                        ps[:, :],
                        kT[h][:, kv * P:(kv + 1) * P],
                        qT[h][:, 0:QW],
                        start=True, stop=True,
                    )
                    ex = exl.tile([P, QW], bf16, tag="ex",
                                  name=f"ex_{qc}_{h}_{kv}")
                    nc.scalar.activation(ex[:, :], ps[:, :], AF.Exp, scale=SCALE)
                    return ex

                # One flat 64-iteration kv stream across all 4 heads with a
                # 2-deep exp pipeline crossing head boundaries, so PE never
                # waits on the first exps of a new head.
                po = exsumA = exsumB = None
                NIT = HPG * KVC
                exq = [scores_mm(0, 0), scores_mm(0, 1)]
                for hk in range(NIT):
                    h, kv = divmod(hk, KVC)
                    if kv == 0:
                        po = psO.tile([P, QW], f32, tag="po",
                                      name=f"po_{qc}_{h}")
                        # Two partial exp-sums: even kv chunks on DVE, odd
                        # on Pool, so neither engine gates the PE.
                        exsumA = exs.tile([P, QW], f32, tag="exsumA",
                                          name=f"exsumA_{qc}_{h}")
                        exsumB = exs.tile([P, QW], f32, tag="exsumB",
                                          name=f"exsumB_{qc}_{h}")
                    ex = exq.pop(0)
                    nc.tensor.matmul(
                        po[:, :],
                        vS[kv][:, h * D:(h + 1) * D],
                        ex[:, :],
                        start=(kv == 0), stop=(kv == KVC - 1),
                    )
                    if kv == 0:
                        nc.vector.tensor_copy(exsumA[:, :], ex[:, :])
                    elif kv == 1:
                        nc.gpsimd.tensor_copy(exsumB[:, :], ex[:, :])
                    elif kv % 2 == 0:
                        nc.vector.tensor_add(exsumA[:, :], exsumA[:, :],
                                             ex[:, :])
                    else:
                        nc.gpsimd.tensor_add(exsumB[:, :], exsumB[:, :],
                                             ex[:, :])
                    fill()
                    if hk + 2 < NIT:
                        h2, kv2 = divmod(hk + 2, KVC)
                        exq.append(scores_mm(h2, kv2))
                    fill()
                    if kv == 1 and pending_norm:
                        pending_norm.pop(0)()
                    if kv == KVC - 1:
                        def make_norm(h=h, po=po, exsumA=exsumA,
                                      exsumB=exsumB):
                            def _norm():
                                sumb = rcp.tile([P, QW], f32, tag="sumb",
                                                name=f"sumb_{qc}_{h}")
                                recb = rcp.tile([P, QW], f32, tag="recb",
                                                name=f"recb_{qc}_{h}")
                                nc.vector.tensor_add(
                                    exsumB[:, :], exsumB[:, :], exsumA[:, :])
                                nc.gpsimd.partition_all_reduce(
                                    sumb[:, :], exsumB[:, :], 128,
                                    ReduceOp.add)
                                nc.vector.reciprocal(recb[:, :], sumb[:, :])
                                nc.vector.tensor_mul(ao[h][:, :], po[:, :],
                                                     recb[:, :])
                            return _norm
                        pending_norm.append(make_norm())

                for fn in pending_norm:
                    fn()
                return ao

            # Prologue: Q projection for chunk 0 runs standalone.
            qT_cur = new_qT(0)
            for _ in qproj_gen(0, hsb0, qT_cur):
                pass
            hsb = load_hsB(1)

            prev_ao = None
            prev_qc = None
            for qc in range(NQC):
                opj = opj_gen(prev_qc, prev_ao) if prev_ao is not None else None
                if qc + 1 < NQC:
                    qT_next = new_qT(qc + 1)
                    qpj = qproj_gen(qc + 1, hsb, qT_next)
                else:
                    qT_next, qpj = None, None
                ao = attention(qc, qT_cur, slot_list(opj, qpj))
                prev_ao, prev_qc = ao, qc
                qT_cur = qT_next
                if qc + 2 < NQC:
                    hsb = load_hsB(qc + 2)
            # tail: out-projection of the last chunk
            opj_tail(prev_qc, prev_ao)

    nc.compile()
    return nc


def get_nc():
    if "nc" not in _NC_CACHE:
        _NC_CACHE["nc"] = _build_nc()
    return _NC_CACHE["nc"]


def _rope_tables():
    inv_freq = (1.0 / (10000.0 ** (np.arange(0, D, 2, dtype=np.float32) / np.float32(D)))).astype(np.float32)
    t = np.arange(S, dtype=np.float32)
    freqs = t[:, None] * inv_freq[None, :]               # [S, 64]
    emb = np.concatenate([freqs, freqs], axis=1)         # [S, 128]
    cosT = np.ascontiguousarray(np.cos(emb).T.astype(np.float32))      # [128, S]
    sin = np.sin(emb).astype(np.float32)                 # [S, 128]
    sinTs = np.ascontiguousarray(
        np.concatenate([-sin[:, :64], sin[:, 64:]], axis=1).T.astype(np.float32)
    )                                                    # [128, S]
    return cosT, sinTs


def _bf16(x):
    import ml_dtypes
    return np.ascontiguousarray(x).astype(ml_dtypes.bfloat16)


def make_in_maps(hidden_states, Wq, bq, Wk, bk, Wv, bv, Wo, bo):
    cosT, sinTs = _rope_tables()
    hsT = [_bf16(np.asarray(hidden_states[b], dtype=np.float32).T) for b in range(B)]
    in_maps = []
    for c in range(8):
        b, g = divmod(c, G)
        gs = slice(g * F, (g + 1) * F)
        in_maps.append({
            "hsT": hsT[b],
            "wqT": _bf16(np.asarray(Wq)[gs, :].T),
            "wkT": _bf16(np.asarray(Wk)[gs, :].T),
            "wvT": _bf16(np.asarray(Wv)[gs, :].T),
            "woT": _bf16(np.asarray(Wo)[:, gs].T),
            "bq2": np.ascontiguousarray(
                np.asarray(bq, dtype=np.float32)[gs].reshape(HPG, P).T),
            "bk2": np.ascontiguousarray(
                np.asarray(bk, dtype=np.float32)[gs].reshape(HPG, P).T),
            "cosT": cosT,
            "sinR": np.ascontiguousarray(np.roll(sinTs, 64, axis=0)),
        })
    return in_maps


def assemble_output(results, bv, Wo, bo):
    out = np.zeros((B, S, E), dtype=np.float32)
    for c in range(8):
        b = c // G
        out[b] += results[c]["out"]
    # v-bias folded out on device: softmax rows sum to 1, so the bv term is
    # the constant row bv @ Wo^T; add it with bo here.
    const_row = (np.asarray(bv, dtype=np.float32)
                 @ np.asarray(Wo, dtype=np.float32).T
                 + np.asarray(bo, dtype=np.float32))
    out += const_row[None, None, :]
    return out


def run_with_results(inputs, trace=False, **trace_kwargs):
    from concourse.bass_utils import run_bass_kernel_spmd
    nc = get_nc()
    in_maps = make_in_maps(**inputs)
    res = run_bass_kernel_spmd(nc, in_maps, list(range(8)), trace=trace, **trace_kwargs)
    out = assemble_output(res.results, inputs["bv"], inputs["Wo"], inputs["bo"])
    return out, res


def kernel(**inputs):
    out, _ = run_with_results(inputs)
    return out


# revision 33
# speedup vs baseline: 1.0597x; 1.0009x over previous
"""Bass/Tile TRN2 kernel for LunarAttention (RoPE attention, B=2, S=2048, E=2048, H=16).

Sharding: 8 cores = 2 batches x 4 head-groups (4 heads / 512 dims each).
Per core (batch b, head-group g):
  Phase A: kT = rope(Wk_g hs^T + bk) in [d, s]; vS = hs Wv_g^T in [s, d]
           (v bias handled host-side: softmax weights sum to 1, so the bv
            term reduces to a constant bv_g @ Wo_g^T row added at gather).
  Phase B per 512-wide q chunk: q projection + rope, then per head
           scoresT[kv,q] -> exp (scalar engine) -> attn@v accumulation on PE.
           Softmax denominators via Pool-engine accumulation of exp tiles +
           partition_all_reduce (no PE ones-matmuls). Out-projection of the
           previous q chunk is interleaved into the attention loop.
dtypes: hs/W in bf16 (host-cast), attention math fp32/f32r, PSUM fp32.
"""

import math
import numpy as np

B, S, E, H, D = 2, 2048, 2048, 16, 128
G = 4            # head groups (cores per batch)
HPG = 4          # heads per group
F = HPG * D      # 512 dims per group
P = 128
EC = E // P      # 16 contraction chunks for projections
SB = 512         # phase-A s-block width
NSB = S // SB    # 4
QW = 512         # phase-B q chunk width
NQC = S // QW    # 4
KVC = S // P     # 16 kv chunks
NEC = 4          # out-proj e chunks of 512
SCALE = 1.0 / math.sqrt(D)

_NC_CACHE = {}


def _build_nc():
    import concourse.bass as bass
    import concourse.mybir as mybir
    import concourse.tile as tile
    from concourse import bacc
    from concourse import bass_isa
    from contextlib import ExitStack

    f32 = mybir.dt.float32
    f32r = mybir.dt.float32r
    bf16 = mybir.dt.bfloat16
    AF = mybir.ActivationFunctionType

    def r(ap):
        return ap.bitcast(f32r)

    nc = bacc.Bacc("TRN2", target_bir_lowering=False)

    hsT_d = nc.dram_tensor("hsT", [E, S], bf16, kind="ExternalInput")
    wqT_d = nc.dram_tensor("wqT", [E, F], bf16, kind="ExternalInput")
    wkT_d = nc.dram_tensor("wkT", [E, F], bf16, kind="ExternalInput")
    wvT_d = nc.dram_tensor("wvT", [E, F], bf16, kind="ExternalInput")
    woT_d = nc.dram_tensor("woT", [F, E], bf16, kind="ExternalInput")
    bq_d = nc.dram_tensor("bq2", [P, HPG], f32, kind="ExternalInput")
    bk_d = nc.dram_tensor("bk2", [P, HPG], f32, kind="ExternalInput")
    cos_d = nc.dram_tensor("cosT", [P, S], f32, kind="ExternalInput")
    sin_d = nc.dram_tensor("sinR", [P, S], f32, kind="ExternalInput")
    out_d = nc.dram_tensor("out", [S, E], f32, kind="ExternalOutput")

    hsT_r = hsT_d[:, :].rearrange("(c p) s -> p c s", p=P)      # [128, 16, 2048]
    wqT_r = wqT_d[:, :].rearrange("(c p) f -> p c f", p=P)
    wkT_r = wkT_d[:, :].rearrange("(c p) f -> p c f", p=P)
    wvT_r = wvT_d[:, :].rearrange("(c p) f -> p c f", p=P)
    woT_r = woT_d[:, :].rearrange("(c p) e -> p c e", p=P)      # [128, 4, 2048]

    with tile.TileContext(nc) as tc, ExitStack() as ctx:
        const = ctx.enter_context(tc.tile_pool(name="const", bufs=1))
        persist = ctx.enter_context(tc.tile_pool(name="persist", bufs=1))
        wpool = ctx.enter_context(tc.tile_pool(name="wpool", bufs=1))

        cos_sb = const.tile([P, S], f32, tag="cos")
        sin_sb = const.tile([P, S], f32, tag="sin")
        bq_sb = const.tile([P, HPG], f32, tag="bq")
        bk_sb = const.tile([P, HPG], f32, tag="bk")

        wq_sb = wpool.tile([P, EC, F], bf16, tag="wq")
        wk_sb = wpool.tile([P, EC, F], bf16, tag="wk")
        wv_sb = wpool.tile([P, EC, F], bf16, tag="wv")
        wo_sb = wpool.tile([P, HPG, E], bf16, tag="wo")

        # The first hs block and wk are the only DMAs the first K matmuls
        # wait on; all other loads are deferred to later s-blocks and issued
        # on the Activation DGE queue so DMA-engine arbitration cannot put
        # them ahead of the critical pair.
        def _load_trig_chunk(sb):
            # cos/sin consumers are per 512-wide block; loading per block
            # keeps the startup DMA window to ~1.5us of trig traffic.
            csl = slice(sb * SB, (sb + 1) * SB)
            nc.scalar.dma_start(out=cos_sb[:, csl], in_=cos_d[:, csl])
            nc.scalar.dma_start(out=sin_sb[:, csl], in_=sin_d[:, csl])

        def _deferred_consts_sb1():
            nc.scalar.dma_start(out=bq_sb[:, :], in_=bq_d[:, :])
            nc.scalar.dma_start(out=wq_sb[:, :, :], in_=wqT_r[:, :, :])
            nc.scalar.dma_start(out=wo_sb[:, :, :], in_=woT_r[:, :, :])

        kT = [persist.tile([P, S], f32r, tag=f"kT{h}", name=f"kT{h}")
              for h in range(HPG)]
        vS = [persist.tile([P, F], bf16, tag=f"vS{i}", name=f"vS{i}")
              for i in range(S // P)]

        def rope_halves(raw, dstT, dsl, psl, rp, sb, fc, nm):
            """dstT[:, dsl] = raw*cos[psl] + rotate_half(raw)*sin[psl], on DVE
            via cross-partition reads (no DMA swaps). sinTs rows 0:64 hold
            -sin, 64:128 hold +sin. DVE op cost scales with free size only,
            so the two half-partition muls cost the same as full ones."""
            w = dsl.stop - dsl.start
            ta = rp.tile([P, w], f32, tag=f"ta{nm}", name=f"ta_{nm}_{sb}_{fc}")
            tb = rp.tile([P, w], f32, tag=f"tb{nm}", name=f"tb_{nm}_{sb}_{fc}")
            nc.vector.tensor_mul(ta[:, :], raw[:, :], cos_sb[:, psl])
            # sin_sb holds sinTs rolled by 64 partitions so each mul's two
            # inputs share a base partition (HW verifier NCC_IBIR297); only
            # the output is partition-shifted.
            nc.vector.tensor_mul(tb[0:64, :], raw[64:128, :], sin_sb[64:128, psl])
            nc.vector.tensor_mul(tb[64:128, :], raw[0:64, :], sin_sb[0:64, psl])
            nc.vector.tensor_add(dstT[:, dsl], ta[:, :], tb[:, :])

        # Phase-B hs pool hoisted so chunk 0 can prefetch during phase A.
        hspB = ctx.enter_context(tc.tile_pool(name="hspB", bufs=1))

        def load_hsB(qc):
            qsl = slice(qc * QW, (qc + 1) * QW)
            hs_q = hspB.tile([P, EC, QW], bf16, tag="hsb", name=f"hsB_{qc}")
            nc.sync.dma_start(out=hs_q[:, :, :], in_=hsT_r[:, :, qsl])
            return hs_q

        # ---- Phase A: K (rope) + V projections ----
        with tc.tile_pool(name="hspA", bufs=2) as hspA, \
             tc.tile_pool(name="ppk", bufs=2, space="PSUM") as ppk, \
             tc.tile_pool(name="ppv", bufs=2, space="PSUM") as ppv, \
             tc.tile_pool(name="rpk", bufs=2) as rpk:
            for sb in range(NSB):
                ssl = slice(sb * SB, (sb + 1) * SB)
                hs_sb = hspA.tile([P, EC, SB], bf16, tag="hs", name=f"hsA_{sb}")
                if sb == 0:
                    # Quarter-granular interleaved loads: the first K matmul
                    # only waits for the first wk/hs chunk pair (~1.5us of
                    # DMA), not the full 4MB. Emission order must match
                    # producer->consumer order; the Act DGE queue keeps the
                    # const loads off the SP-queue semaphore gating the
                    # first K matmuls.
                    for cc in range(4):
                        ecs = slice(cc * 4, (cc + 1) * 4)
                        nc.sync.dma_start(out=wk_sb[:, ecs, :],
                                          in_=wkT_r[:, ecs, :])
                        nc.sync.dma_start(out=hs_sb[:, ecs, :],
                                          in_=hsT_r[:, ecs, ssl])
                    nc.scalar.dma_start(out=bk_sb[:, :], in_=bk_d[:, :])
                    _load_trig_chunk(0)
                else:
                    nc.sync.dma_start(out=hs_sb[:, :, :], in_=hsT_r[:, :, ssl])
                    _load_trig_chunk(sb)
                    if sb == 1:
                        _deferred_consts_sb1()
                for fc in range(HPG):
                    psK = ppk.tile([P, SB], f32, tag="psk", name=f"psk_{sb}_{fc}")
                    for ec in range(EC):
                        nc.tensor.matmul(
                            psK[:, :],
                            wk_sb[:, ec, fc * P:(fc + 1) * P],
                            hs_sb[:, ec, :],
                            start=(ec == 0),
                            stop=(ec == EC - 1),
                        )
                    raw = rpk.tile([P, SB], f32, tag="rawk", name=f"rawk_{sb}_{fc}")
                    nc.scalar.add(raw[:, :], psK[:, :], bk_sb[:, fc:fc + 1])
                    rope_halves(raw, kT[fc], ssl, ssl, rpk, sb, fc, "k")
                if sb == 0:
                    nc.scalar.dma_start(out=wv_sb[:, :, :], in_=wvT_r[:, :, :])
                for m in range(SB // P):
                    psV = ppv.tile([P, F], f32, tag="psv", name=f"psv_{sb}_{m}")
                    for ec in range(EC):
                        nc.tensor.matmul(
                            psV[:, :],
                            hs_sb[:, ec, m * P:(m + 1) * P],
                            wv_sb[:, ec, :],
                            start=(ec == 0),
                            stop=(ec == EC - 1),
                        )
                    nc.scalar.copy(vS[sb * (SB // P) + m][:, :], psV[:, :])

        # ---- Phase B: Q proj + attention + interleaved out-projection ----
        # Schedule: per q chunk, the attention loop (64 kv iterations) has
        # two PE filler slots per iteration, fed by the previous chunk's
        # out-projection matmuls (64) and the next chunk's Q-projection
        # matmuls (64). This keeps PE fed past the Act-bound exp stream and
        # removes standalone projection phases where Act would idle.
        from concourse.bass_isa import ReduceOp
        hsb0 = load_hsB(0)
        with tc.tile_pool(name="qtp", bufs=2) as qtp, \
             tc.tile_pool(name="rpq", bufs=2) as rpq, \
             tc.tile_pool(name="psQ", bufs=2, space="PSUM") as psQ, \
             tc.tile_pool(name="psS", bufs=2, space="PSUM") as psS, \
             tc.tile_pool(name="psO", bufs=2, space="PSUM") as psO, \
             tc.tile_pool(name="psP", bufs=2, space="PSUM") as psP, \
             tc.tile_pool(name="exl", bufs=6) as exl, \
             tc.tile_pool(name="exs", bufs=2) as exs, \
             tc.tile_pool(name="rcp", bufs=2) as rcp, \
             tc.tile_pool(name="aop", bufs=2) as aop, \
             tc.tile_pool(name="otp", bufs=2) as otp:

            def qproj_gen(qc, hs_q, qT):
                """Yield once per Q-projection matmul for chunk qc; bias-add
                and rope are emitted inline as each head-part completes."""
                qsl = slice(qc * QW, (qc + 1) * QW)
                for fc in range(HPG):
                    ps_q = psQ.tile([P, QW], f32, tag="q", name=f"psq_{qc}_{fc}")
                    for ec in range(EC):
                        nc.tensor.matmul(
                            ps_q[:, :],
                            wq_sb[:, ec, fc * P:(fc + 1) * P],
                            hs_q[:, ec, :],
                            start=(ec == 0),
                            stop=(ec == EC - 1),
                        )
                        if ec < EC - 1:
                            yield
                    rawq = rpq.tile([P, QW], f32, tag="rawq", name=f"rawq_{qc}_{fc}")
                    nc.scalar.add(rawq[:, :], ps_q[:, :], bq_sb[:, fc:fc + 1])
                    rope_halves(rawq, qT[fc], slice(0, QW), qsl, rpq, qc, fc, "q")
                    yield

            def new_qT(qc):
                return [qtp.tile([P, QW], f32r, tag=f"qT{fc}", name=f"qT_{qc}_{fc}")
                        for fc in range(HPG)]

            def opj_gen(qc, ao):
                """Yield once per out-projection matmul for chunk qc
                (interleaved into the next chunk's attention loop)."""
                for ecn in range(NEC):
                    for m in range(QW // P):
                        pp = psP.tile([P, 512], f32, tag="pp",
                                      name=f"pp_{qc}_{ecn}_{m}")
                        for fc in range(HPG):
                            nc.tensor.matmul(
                                pp[:, :],
                                ao[fc][:, m * P:(m + 1) * P],
                                wo_sb[:, fc, ecn * 512:(ecn + 1) * 512],
                                start=(fc == 0), stop=(fc == HPG - 1),
                            )
                            if fc < HPG - 1:
                                yield
                        ot = otp.tile([P, 512], f32, tag="ot",
                                      name=f"ot_{qc}_{ecn}_{m}")
                        nc.vector.tensor_copy(ot[:, :], pp[:, :])
                        nc.sync.dma_start(
                            out=out_d[qc * QW + m * P: qc * QW + (m + 1) * P,
                                      ecn * 512:(ecn + 1) * 512],
                            in_=ot[:, :],
                        )
                        yield

            def opj_tail(qc, ao):
                """Un-interleaved tail out-projection: rotate pp tiles
                through all four (now idle) PSUM pools and defer each
                group's last matmul (which needs the final head's ao, whose
                normalization chain is still in flight) by 6 groups, so the
                PE never waits on it."""
                ppools = [(psP, "pp"), (psQ, "q"), (psO, "po"), (psS, "s")]
                opools = [(otp, "ot"), (rpq, "rawq"), (exs, "exsumA"),
                          (rcp, "sumb")]
                finish = []

                def close_group(item):
                    pp, ecn, m, use_act = item
                    nc.tensor.matmul(
                        pp[:, :],
                        ao[HPG - 1][:, m * P:(m + 1) * P],
                        wo_sb[:, HPG - 1, ecn * 512:(ecn + 1) * 512],
                        start=False, stop=True,
                    )
                    g = ecn * (QW // P) + m
                    opool, otag = opools[g % 4]
                    ot = opool.tile([P, 512], f32, tag=otag,
                                    name=f"otT_{qc}_{ecn}_{m}")
                    if use_act:
                        nc.scalar.copy(ot[:, :], pp[:, :])
                    else:
                        nc.vector.tensor_copy(ot[:, :], pp[:, :])
                    nc.sync.dma_start(
                        out=out_d[qc * QW + m * P: qc * QW + (m + 1) * P,
                                  ecn * 512:(ecn + 1) * 512],
                        in_=ot[:, :],
                    )

                for ecn in range(NEC):
                    for m in range(QW // P):
                        g = ecn * (QW // P) + m
                        pool, ptag = ppools[g % 4]
                        pp = pool.tile([P, 512], f32, tag=ptag,
                                       name=f"pp_{qc}_{ecn}_{m}")
                        for fc in range(HPG - 1):
                            nc.tensor.matmul(
                                pp[:, :],
                                ao[fc][:, m * P:(m + 1) * P],
                                wo_sb[:, fc, ecn * 512:(ecn + 1) * 512],
                                start=(fc == 0), stop=False,
                            )
                        finish.append((pp, ecn, m, g % 2 == 1))
                        if len(finish) > 6:
                            close_group(finish.pop(0))
                        if g >= 8 and finish:
                            close_group(finish.pop(0))
                        if g >= 12 and finish:
                            close_group(finish.pop(0))
                for item in finish:
                    close_group(item)

            _DONE = object()

            def slot_list(opj, qpj):
                """128 lazy PE filler slots for one attention loop: the first
                16 slots draw from the out-projection stream only (lets the
                next hs block land before Q-projection starts), then the two
                streams alternate. Each fill() advances one generator by one
                matmul emission."""
                if opj is not None and qpj is not None:
                    order = []
                    o_left, q_left = 64, 64
                    for _ in range(16):
                        order.append(opj)
                        o_left -= 1
                    while o_left or q_left:
                        if q_left:
                            order.append(qpj)
                            q_left -= 1
                        if o_left:
                            order.append(opj)
                            o_left -= 1
                    return order
                # Single-stream chunks (first: no out-projection; last: no
                # next Q-projection): spread the 64 fills one per iteration,
                # otherwise the last 32 iterations run bare and the PE
                # starves behind the Act exp stream.
                gen = opj if opj is not None else qpj
                if gen is None:
                    return []
                return [gen, None] * 64

            def attention(qc, qT, slots):
                """Attention for chunk qc; two filler thunks per kv iter."""
                ao = [aop.tile([P, QW], mybir.dt.bfloat16, tag=f"ao{h}",
                               name=f"ao_{qc}_{h}") for h in range(HPG)]
                pending_norm = []
                si = iter(slots)

                def fill():
                    gen = next(si, None)
                    if gen is not None:
                        next(gen, None)

                def scores_mm(h, kv):
                    ps = psS.tile([P, QW], f32, tag="s", name=f"ps_{qc}_{h}_{kv}")
                    nc.tensor.matmul(
                        ps[:, :],
                        kT[h][:, kv * P:(kv + 1) * P],
                        qT[h][:, 0:QW],
                        start=True, stop=True,
                    )
                    ex = exl.tile([P, QW], bf16, tag="ex",
                                  name=f"ex_{qc}_{h}_{kv}")
                    nc.scalar.activation(ex[:, :], ps[:, :], AF.Exp, scale=SCALE)
                    return ex

                # One flat 64-iteration kv stream across all 4 heads with a
                # 2-deep exp pipeline crossing head boundaries, so PE never
                # waits on the first exps of a new head.
                po = exsumA = exsumB = None
                NIT = HPG * KVC
                exq = [scores_mm(0, 0), scores_mm(0, 1)]
                for hk in range(NIT):
                    h, kv = divmod(hk, KVC)
                    if kv == 0:
                        po = psO.tile([P, QW], f32, tag="po",
                                      name=f"po_{qc}_{h}")
                        # Two partial exp-sums: even kv chunks on DVE, odd
                        # on Pool, so neither engine gates the PE.
                        exsumA = exs.tile([P, QW], f32, tag="exsumA",
                                          name=f"exsumA_{qc}_{h}")
                        exsumB = exs.tile([P, QW], f32, tag="exsumB",
                                          name=f"exsumB_{qc}_{h}")
                    ex = exq.pop(0)
                    nc.tensor.matmul(
                        po[:, :],
                        vS[kv][:, h * D:(h + 1) * D],
                        ex[:, :],
                        start=(kv == 0), stop=(kv == KVC - 1),
                    )
                    if kv == 0:
                        nc.vector.tensor_copy(exsumA[:, :], ex[:, :])
                    elif kv == 1:
                        nc.gpsimd.tensor_copy(exsumB[:, :], ex[:, :])
                    elif kv % 2 == 0:
                        nc.vector.tensor_add(exsumA[:, :], exsumA[:, :],
                                             ex[:, :])
                    else:
                        nc.gpsimd.tensor_add(exsumB[:, :], exsumB[:, :],
                                             ex[:, :])
                    fill()
                    if hk + 2 < NIT:
                        h2, kv2 = divmod(hk + 2, KVC)
                        exq.append(scores_mm(h2, kv2))
                    fill()
                    if kv == 1 and pending_norm:
                        pending_norm.pop(0)()
                    if kv == KVC - 1:
                        def make_norm(h=h, po=po, exsumA=exsumA,
                                      exsumB=exsumB):
                            def _norm():
                                sumb = rcp.tile([P, QW], f32, tag="sumb",
                                                name=f"sumb_{qc}_{h}")
                                recb = rcp.tile([P, QW], f32, tag="recb",
                                                name=f"recb_{qc}_{h}")
                                nc.vector.tensor_add(
                                    exsumB[:, :], exsumB[:, :], exsumA[:, :])
                                nc.gpsimd.partition_all_reduce(
                                    sumb[:, :], exsumB[:, :], 128,
                                    ReduceOp.add)
                                nc.vector.reciprocal(recb[:, :], sumb[:, :])
                                nc.vector.tensor_mul(ao[h][:, :], po[:, :],
                                                     recb[:, :])
                            return _norm
                        pending_norm.append(make_norm())

                for fn in pending_norm:
                    fn()
                return ao

            # Prologue: Q projection for chunk 0 runs standalone.
            qT_cur = new_qT(0)
            for _ in qproj_gen(0, hsb0, qT_cur):
                pass
            hsb = load_hsB(1)

            prev_ao = None
            prev_qc = None
            for qc in range(NQC):
                opj = opj_gen(prev_qc, prev_ao) if prev_ao is not None else None
                if qc + 1 < NQC:
                    qT_next = new_qT(qc + 1)
                    qpj = qproj_gen(qc + 1, hsb, qT_next)
                else:
                    qT_next, qpj = None, None
                ao = attention(qc, qT_cur, slot_list(opj, qpj))
                prev_ao, prev_qc = ao, qc
                qT_cur = qT_next
                if qc + 2 < NQC:
                    hsb = load_hsB(qc + 2)
            # tail: out-projection of the last chunk
            opj_tail(prev_qc, prev_ao)

    nc.compile()
    return nc


def get_nc():
    if "nc" not in _NC_CACHE:
        _NC_CACHE["nc"] = _build_nc()
    return _NC_CACHE["nc"]


def _rope_tables():
    inv_freq = (1.0 / (10000.0 ** (np.arange(0, D, 2, dtype=np.float32) / np.float32(D)))).astype(np.float32)
    t = np.arange(S, dtype=np.float32)
    freqs = t[:, None] * inv_freq[None, :]               # [S, 64]
    emb = np.concatenate([freqs, freqs], axis=1)         # [S, 128]
    cosT = np.ascontiguousarray(np.cos(emb).T.astype(np.float32))      # [128, S]
    sin = np.sin(emb).astype(np.float32)                 # [S, 128]
    sinTs = np.ascontiguousarray(
        np.concatenate([-sin[:, :64], sin[:, 64:]], axis=1).T.astype(np.float32)
    )                                                    # [128, S]
    return cosT, sinTs


def _bf16(x):
    import ml_dtypes
    return np.ascontiguousarray(x).astype(ml_dtypes.bfloat16)


def make_in_maps(hidden_states, Wq, bq, Wk, bk, Wv, bv, Wo, bo):
    cosT, sinTs = _rope_tables()
    hsT = [_bf16(np.asarray(hidden_states[b], dtype=np.float32).T) for b in range(B)]
    in_maps = []
    for c in range(8):
        b, g = divmod(c, G)
        gs = slice(g * F, (g + 1) * F)
        in_maps.append({
            "hsT": hsT[b],
            "wqT": _bf16(np.asarray(Wq)[gs, :].T),
            "wkT": _bf16(np.asarray(Wk)[gs, :].T),
            "wvT": _bf16(np.asarray(Wv)[gs, :].T),
            "woT": _bf16(np.asarray(Wo)[:, gs].T),
            "bq2": np.ascontiguousarray(
                np.asarray(bq, dtype=np.float32)[gs].reshape(HPG, P).T),
            "bk2": np.ascontiguousarray(
                np.asarray(bk, dtype=np.float32)[gs].reshape(HPG, P).T),
            "cosT": cosT,
            "sinR": np.ascontiguousarray(np.roll(sinTs, 64, axis=0)),
        })
    return in_maps


def assemble_output(results, bv, Wo, bo):
    out = np.zeros((B, S, E), dtype=np.float32)
    for c in range(8):
        b = c // G
        out[b] += results[c]["out"]
    # v-bias folded out on device: softmax rows sum to 1, so the bv term is
    # the constant row bv @ Wo^T; add it with bo here.
    const_row = (np.asarray(bv, dtype=np.float32)
                 @ np.asarray(Wo, dtype=np.float32).T
                 + np.asarray(bo, dtype=np.float32))
    out += const_row[None, None, :]
    return out


def run_with_results(inputs, trace=False, **trace_kwargs):
    from concourse.bass_utils import run_bass_kernel_spmd
    nc = get_nc()
    in_maps = make_in_maps(**inputs)
    res = run_bass_kernel_spmd(nc, in_maps, list(range(8)), trace=trace, **trace_kwargs)
    out = assemble_output(res.results, inputs["bv"], inputs["Wo"], inputs["bo"])
    return out, res


def kernel(**inputs):
    out, _ = run_with_results(inputs)
    return out
